# revision 23
# baseline (speedup 1.0000x reference)
"""MultiHeadEMABlock Trainium2 kernel v6 (8-core SPMD, bass/Tile).

Math: out = x + R(EMA_bank(LN(x))) where the 8-head EMA+reduction kernel
bank K[d, tau] = sum_h rho[h,d] q_h^tau (rho = a*e*r*gamma) is numerically
rank-3 (q in [0.026, 0.57]; svals 36.7/5.3/0.83/0.13...). We compute
R=3 pseudo-heads: K ~= sum_r u_r[d] v_r(tau).

Per core (4 batches x 2 halves; halo W=128; q^128 ~ 1e-31 so no carry):
  chunk ck (128 timesteps):
   1. GpSimd: cast x f32->bf16
   2. PE: 4 ident-transpose matmuls -> x^T psum; +1-col matmuls -> Sx
   3. ACT: Square w/ accum_out -> Sx2 (scrap squares discarded)
   4. (per 2 chunks) LN finalize: rstd=exp(-.5 ln(var+eps)), negms=-m*rstd
   5. ACT: z^T = Identity(x^T * rstd + negms)  [per-partition affine]
   6. DVE/GpSimd: X_r = z^T * u_r (bf16 tensor_tensor, 3 ops)
   7. PE: 3 main T'_r matmuls (lower-tri v_r(i-j)) + 3 edge matmuls
      (last 32 rows of prev chunk's X_r, lags 1..63) accumulate in psum
   8. DVE: psum -> bf16 (per chunk pair), DMA out TIME-MAJOR [NHALF, C]
  Host: transpose back, add residual x and the data-independent beta term.
"""
import contextlib
import ctypes
import sys
import types

import numpy as np

for _p in ("/root/.axon_site/_ro/trn_rl_repo", "/opt/trn_rl_repo"):
    if _p not in sys.path:
        sys.path.append(_p)

B, C, N, H = 4, 512, 4096, 8
EPS = 1e-5
N_CORES = 8
NHALF = N // 2
W = 128
NW = NHALF + W
L = 128
NCH = NW // L  # 17
CT = C // 128  # 4
R = 3


# ---------------------------------------------------------------------------
# axon NTFF shim (lets run_bass_kernel_spmd(trace=True) capture HW profiles)
# ---------------------------------------------------------------------------
def _install_ntff_shim():
    if "antenv.axon_hooks" in sys.modules:
        return
    holder = {"hook": None}

    def _make(so_path):
        try:
            lib = ctypes.CDLL(so_path)
        except OSError:
            return None
        if not hasattr(lib, "axon_start_nrt_profile"):
            return None
        lib.axon_start_nrt_profile.argtypes = [
            ctypes.POINTER(ctypes.c_int64),
            ctypes.c_size_t,
        ]
        lib.axon_start_nrt_profile.restype = ctypes.c_int64
        lib.axon_stop_nrt_profile.argtypes = [ctypes.c_char_p]
        lib.axon_stop_nrt_profile.restype = ctypes.c_int64

        @contextlib.contextmanager
        def _hook(output_dir, device_ids):
            import jax

            jax.devices()
            if device_ids:
                ids = (ctypes.c_int64 * len(device_ids))(*device_ids)
                rc = lib.axon_start_nrt_profile(ids, len(device_ids))
            else:
                rc = lib.axon_start_nrt_profile(None, 0)
            if rc != 0:
                raise RuntimeError(f"axon_start_nrt_profile rc={rc}")
            try:
                yield
            finally:
                n = lib.axon_stop_nrt_profile(str(output_dir).encode())
                print(f"ntff profile: {n} file(s) -> {output_dir}", file=sys.stderr)

        return _hook

    mod = types.ModuleType("antenv.axon_hooks")
    mod.set_axon_ntff_profile_hook = lambda h: holder.__setitem__("hook", h)
    mod.get_axon_ntff_profile_hook = lambda: holder["hook"]
    sys.modules["antenv.axon_hooks"] = mod
    try:
        import antenv

        antenv.axon_hooks = mod
    except ImportError:
        pass
    holder["hook"] = _make("/opt/axon/libaxon_pjrt.so")


def _split_multiwait(nc, max_waits=1):
    """This walrus build rejects >1 sync wait per instruction; split extras
    onto same-engine NoOps inserted just before."""
    from concourse import mybir

    k = [0]
    for fn in nc.m.functions:
        for blk in fn.blocks:
            out = []
            for inst in blk.instructions:
                si = getattr(inst, "sync_info", None)
                if si is not None and len(si.on_wait) > max_waits:
                    waits = list(si.on_wait)
                    for w in waits[max_waits:]:
                        k[0] += 1
                        out.append(
                            mybir.InstNoOp(
                                name=f"{inst.name}-mw{k[0]}",
                                sync_info=mybir.SyncInfo(on_wait=[w], on_update=[]),
                                bass_nofuse=True,
                                engine=inst.engine,
                            )
                        )
                    inst.sync_info = mybir.SyncInfo(
                        on_wait=waits[:max_waits], on_update=list(si.on_update)
                    )
                out.append(inst)
            blk.instructions[:] = out


# ---------------------------------------------------------------------------
# program builder
# ---------------------------------------------------------------------------
def build_program():
    import concourse.bass as bass
    import concourse.tile as tile
    from concourse import mybir

    f32 = mybir.dt.float32
    bf16 = mybir.dt.bfloat16
    Op = mybir.AluOpType
    Act = mybir.ActivationFunctionType

    # 512-wide input DMA slices
    stat_slices = []
    o = 0
    while o < NW:
        w = min(512, NW - o)
        stat_slices.append((o, w))
        o += w

    # stat finalize groups (pairs)
    groups = [(k, min(k + 2, NCH)) for k in range(0, NCH, 2)]
    group_end = {g1 - 1: (g0, g1) for g0, g1 in groups}

    nc = bass.Bass(
        "TRN2",
        target_bir_lowering=False,
        debug=False,
        enable_asserts=False,
        num_devices=N_CORES,
    )
    xs_d = nc.dram_tensor("xs", [C, NW], f32, kind="ExternalInput").ap()
    tm_d = nc.dram_tensor("tmat", [R * 128, 128], bf16, kind="ExternalInput").ap()
    te_d = nc.dram_tensor("temat", [R * 32, 32], bf16, kind="ExternalInput").ap()
    ub_d = nc.dram_tensor("ubc", [R * 128, 512], bf16, kind="ExternalInput").ap()
    id_d = nc.dram_tensor("ident", [128, 128], bf16, kind="ExternalInput").ap()
    oc_d = nc.dram_tensor("onecol", [128, 1], bf16, kind="ExternalInput").ap()
    out_d = nc.dram_tensor("out_t", [NHALF, C], bf16, kind="ExternalOutput").ap()

    with tile.TileContext(nc) as tc:
        with contextlib.ExitStack() as ctx:
            pers = ctx.enter_context(tc.tile_pool(name="pers", bufs=1))
            xb_pool = ctx.enter_context(tc.tile_pool(name="xbp", bufs=4))
            zx_pool = ctx.enter_context(tc.tile_pool(name="zxp", bufs=3))
            ep_pool = ctx.enter_context(tc.tile_pool(name="epp", bufs=2))
            sq_pool = ctx.enter_context(tc.tile_pool(name="sqp", bufs=2))
            osb_pool = ctx.enter_context(tc.tile_pool(name="osbp", bufs=3))
            ps_t = ctx.enter_context(tc.tile_pool(name="pst", bufs=5, space="PSUM"))
            ps_o = ctx.enter_context(tc.tile_pool(name="pso", bufs=2, space="PSUM"))
            ps_s = ctx.enter_context(tc.tile_pool(name="pss", bufs=1, space="PSUM"))

            # ---- constants (scalar HWDGE queue; input on sync queue) ----
            ident = pers.tile([128, 128], bf16, tag="ident")
            nc.scalar.dma_start(out=ident[:], in_=id_d)
            onec = pers.tile([128, 1], bf16, tag="onec")
            nc.scalar.dma_start(out=onec[:], in_=oc_d)
            tm_t = [pers.tile([128, 128], bf16, tag=f"tm{r}", name=f"tm{r}")
                    for r in range(R)]
            for r in range(R):
                nc.scalar.dma_start(out=tm_t[r][:], in_=tm_d[r * 128:(r + 1) * 128, :])
            # packed edge stationary [3*32, 32] at partitions 0:96
            teP = pers.tile([96, 32], bf16, tag="teP")
            nc.scalar.dma_start(out=teP[:], in_=te_d)
            ub_t = [pers.tile([128, 512], bf16, tag=f"ub{r}", name=f"ub{r}")
                    for r in range(R)]
            for r in range(R):
                nc.scalar.dma_start(out=ub_t[r][:], in_=ub_d[r * 128:(r + 1) * 128, :])
            epsb = pers.tile([128, 1], f32, tag="eps")
            nc.gpsimd.memset(epsb[:], EPS)
            warm = pers.tile([128, 1], f32, tag="warm")
            nc.scalar.activation(out=warm[:], in_=epsb[:], func=Act.Square)

            s2sb = pers.tile([128, NCH], f32, tag="s2sb")
            m2s = pers.tile([128, NCH], f32, tag="m2s")
            var_a = pers.tile([128, NCH], f32, tag="var")
            lnv_a = pers.tile([128, NCH], f32, tag="lnv")
            rstd_a = pers.tile([128, NCH], f32, tag="rstd")
            negm_a = pers.tile([128, NCH], f32, tag="negm")

            # ---- input: slice 0 split per chunk (ramp priority), rest 512 ----
            xsl = {}
            for si, (o, wd) in enumerate(stat_slices):
                t = pers.tile([128, CT * 512], f32, tag=f"xsl{si}",
                              name=f"xsl{si}")
                xsl[si] = t
                if si == 0:
                    for cc in range(4):
                        for dt in range(CT):
                            nc.sync.dma_start(
                                out=t[:, dt * 512 + cc * L: dt * 512 + (cc + 1) * L],
                                in_=xs_d[dt * 128:(dt + 1) * 128,
                                         cc * L:(cc + 1) * L],
                            )
                else:
                    for dt in range(CT):
                        nc.sync.dma_start(
                            out=t[:, dt * 512: dt * 512 + wd],
                            in_=xs_d[dt * 128:(dt + 1) * 128, o:o + wd],
                        )

            def phase1(g0, g1):
                """cast, transpose, stat sums for chunks [g0, g1)"""
                out = {}
                statp = state.setdefault(
                    "statp", ps_s.tile([128, 40], f32, tag="stat", name="statp"))
                for ck in range(g0, g1):
                    xb = xb_pool.tile([128, 512], bf16, tag="xb")
                    si, co = divmod(ck * L, 512)
                    nc.vector.tensor_scalar(
                        out=xb[:].rearrange("p (dt i) -> p dt i", dt=CT),
                        in0=xsl[si][:].rearrange("p (dt n) -> p dt n", dt=CT)[
                            :, :, co:co + L],
                        scalar1=1.0, scalar2=None, op0=Op.mult,
                    )
                    xt = ps_t.tile([128, 512], f32, tag="xt")
                    for dt in range(CT):
                        nc.tensor.matmul(
                            out=xt[:, dt * 128:(dt + 1) * 128],
                            lhsT=xb[:, dt * 128:(dt + 1) * 128], rhs=ident[:],
                            start=True, stop=True,
                        )
                    xsq = sq_pool.tile([128, 512], bf16, tag="xsq")
                    nc.vector.tensor_tensor(out=xsq[:], in0=xb[:], in1=xb[:],
                                            op=Op.mult)
                    for dt in range(CT):
                        nc.tensor.matmul(
                            out=statp[:, ck:ck + 1],
                            lhsT=xb[:, dt * 128:(dt + 1) * 128], rhs=onec[:],
                            start=(dt == 0), stop=(dt == CT - 1),
                        )
                    for dt in range(CT):
                        nc.tensor.matmul(
                            out=statp[:, 20 + ck:21 + ck],
                            lhsT=xsq[:, dt * 128:(dt + 1) * 128], rhs=onec[:],
                            start=(dt == 0), stop=(dt == CT - 1),
                        )
                    out[ck] = xt
                return out, statp

            def phase1b(g0, g1, xts):
                pass

            def finalize(g0, g1, statp):
                nc.scalar.activation(
                    out=m2s[:, g0:g1], in_=statp[:, g0:g1], func=Act.Square,
                    scale=1.0 / C,
                )
                nc.vector.scalar_tensor_tensor(
                    out=var_a[:, g0:g1], in0=statp[:, 20 + g0:20 + g1],
                    scalar=1.0 / C,
                    in1=m2s[:, g0:g1], op0=Op.mult, op1=Op.subtract,
                )
                nc.scalar.activation(
                    out=lnv_a[:, g0:g1], in_=var_a[:, g0:g1], func=Act.Ln,
                    bias=epsb[:],
                )
                nc.scalar.activation(
                    out=rstd_a[:, g0:g1], in_=lnv_a[:, g0:g1], func=Act.Exp,
                    scale=-0.5,
                )
                nc.vector.scalar_tensor_tensor(
                    out=negm_a[:, g0:g1], in0=statp[:, g0:g1], scalar=-1.0 / C,
                    in1=rstd_a[:, g0:g1], op0=Op.mult, op1=Op.mult,
                )

            state = {"xc_prev": None, "ep_prev": None}

            def phase2(g0, g1, xts):
                for ck in range(g0, g1):
                    xt = xts[ck]
                    zT = zx_pool.tile([128, 512], bf16, tag="zt")
                    nc.scalar.activation(
                        out=zT[:], in_=xt[:], func=Act.Identity,
                        scale=rstd_a[:, ck:ck + 1], bias=negm_a[:, ck:ck + 1],
                    )
                    x1 = zx_pool.tile([128, 512], bf16, tag="x1")
                    nc.vector.tensor_tensor(out=x1[:], in0=zT[:], in1=ub_t[0][:],
                                            op=Op.mult)
                    x2 = zx_pool.tile([128, 512], bf16, tag="x2")
                    nc.vector.tensor_tensor(out=x2[:], in0=zT[:], in1=ub_t[1][:],
                                            op=Op.mult)
                    x3 = zx_pool.tile([128, 512], bf16, tag="x3")
                    nc.gpsimd.tensor_tensor(out=x3[:], in0=zT[:], in1=ub_t[2][:],
                                            op=Op.mult)
                    xc = [x1, x2, x3]
                    # pack last 32 rows of X_r into partitions [32r, 32r+32)
                    epk = ep_pool.tile([96, 512], bf16, tag="epk")
                    for r in range(R):
                        nc.sync.dma_start(out=epk[r * 32:(r + 1) * 32, :],
                                          in_=xc[r][96:128, :])
                    if ck >= 1:
                        op_ps = ps_o.tile([128, 512], f32, tag="op")
                        nc.tensor.matmul(
                            out=op_ps[:], lhsT=tm_t[0][:], rhs=xc[0][:],
                            start=True, stop=False,
                        )
                        nc.tensor.matmul(
                            out=op_ps[0:32, :], lhsT=teP[:],
                            rhs=state["ep_prev"][:],
                            start=False, stop=False, skip_group_check=True,
                        )
                        nc.tensor.matmul(
                            out=op_ps[:], lhsT=tm_t[1][:], rhs=xc[1][:],
                            start=False, stop=False,
                        )
                        nc.tensor.matmul(
                            out=op_ps[:], lhsT=tm_t[2][:], rhs=xc[2][:],
                            start=False, stop=True,
                        )
                        mo = ck - 1
                        osb = osb_pool.tile([128, 512], bf16, tag="osb")
                        nc.scalar.activation(out=osb[:], in_=op_ps[:],
                                             func=Act.Copy)
                        nc.sync.dma_start(
                            out=out_d[mo * 128:(mo + 1) * 128, :], in_=osb[:])
                    state["xc_prev"] = xc
                    state["ep_prev"] = epk

            # software-pipelined group loop
            prev = None
            for gi, (g0, g1) in enumerate(groups):
                if gi == 0:
                    cur = phase1(g0, g1)
                    phase1b(g0, g1, cur[0])
                    finalize(g0, g1, cur[1])
                    phase2(g0, g1, cur[0])
                    continue
                if prev is not None:
                    pg0, pg1, pxts, pstat = prev
                    finalize(pg0, pg1, pstat)
                    cur = phase1(g0, g1)
                    phase2(pg0, pg1, pxts)
                    phase1b(g0, g1, cur[0])
                else:
                    cur = phase1(g0, g1)
                    phase1b(g0, g1, cur[0])
                prev = (g0, g1, cur[0], cur[1])
            pg0, pg1, pxts, pstat = prev
            finalize(pg0, pg1, pstat)
            phase2(pg0, pg1, pxts)
    return nc


# ---------------------------------------------------------------------------
# host-side parameter prep
# ---------------------------------------------------------------------------
def _host_params(ln_gamma, ln_beta, expansion, reduction, alphas, dampen_factors):
    import ml_dtypes

    bf = ml_dtypes.bfloat16
    a = 1.0 / (1.0 + np.exp(-alphas.astype(np.float64)))
    q = (1.0 - a) / (1.0 + np.exp(-dampen_factors.astype(np.float64)))
    rho = (
        a[:, None]
        * expansion.astype(np.float64)
        * reduction.astype(np.float64)
        * ln_gamma.astype(np.float64)[None, :]
    )  # [H, C]
    tau = np.arange(L)
    K = rho.T @ (q[:, None] ** tau[None, :])  # [C, L]
    U, S, Vt = np.linalg.svd(K, full_matrices=False)
    u = U[:, :R] * S[None, :R]
    v = Vt[:R].copy()
    for r in range(R):
        s = np.sqrt(np.abs(u[:, r]).max() / max(np.abs(v[r]).max(), 1e-30))
        v[r] *= s
        u[:, r] /= s
    vpad = np.concatenate([v, np.zeros((R, 64))], 1)
    tmat = np.zeros((R * 128, 128), bf)
    for r in range(R):
        M = np.where(
            tau[:, None] <= tau[None, :],
            np.take(v[r], np.maximum(tau[None, :] - tau[:, None], 0)), 0.0,
        )  # [j, i] = v_r(i-j), i>=j
        tmat[r * 128:(r + 1) * 128, :] = M.astype(bf)
    jj = np.arange(32)
    lag = jj[None, :] + 32 - jj[:, None]  # [jj, i] in [1, 63]
    temat = np.zeros((R * 32, 32), bf)
    for r in range(R):
        temat[r * 32:(r + 1) * 32, :] = np.take(vpad[r], lag).astype(bf)
    ubc = np.zeros((R * 128, 512), bf)
    for r in range(R):
        ubc[r * 128:(r + 1) * 128, :] = np.tile(
            u[:, r].astype(bf)[None, :], (128, 1))
    consts = dict(
        tmat=tmat, temat=temat, ubc=ubc,
        ident=np.eye(128, dtype=bf),
        onecol=np.ones((128, 1), bf),
    )
    return a, q, consts


def _beta_term(ln_beta, expansion, reduction, a, q):
    if not np.any(ln_beta):
        return None
    n_idx = np.arange(N, dtype=np.float64)
    Cn = a[:, None] * (1.0 - q[:, None] ** (n_idx[None, :] + 1.0)) / (1.0 - q[:, None])
    w = (
        expansion.astype(np.float64)
        * reduction.astype(np.float64)
        * ln_beta.astype(np.float64)[None, :]
    )
    return np.einsum("hc,hn->cn", w, Cn).astype(np.float32)


def _make_in_maps(x, consts):
    in_maps = []
    for core in range(N_CORES):
        b, half = divmod(core, 2)
        xs = np.zeros((C, NW), np.float32)
        s = half * NHALF - W
        if s < 0:
            xs[:, W:] = x[b, :, :NHALF]
        else:
            xs[:] = x[b, :, s:s + NW]
        in_maps.append(dict(consts, xs=xs))
    return in_maps


def kernel(x, ln_gamma, ln_beta, expansion, reduction, alphas, dampen_factors,
           trace=False):
    _install_ntff_shim()
    from concourse.bass_utils import run_bass_kernel_spmd
    from concourse.bass_interp import get_hw_module

    x = np.asarray(x, np.float32)
    a, q, consts = _host_params(
        np.asarray(ln_gamma), np.asarray(ln_beta), np.asarray(expansion),
        np.asarray(reduction), np.asarray(alphas), np.asarray(dampen_factors),
    )
    nc = build_program()
    _split_multiwait(nc)
    nc.m = get_hw_module(nc.m)

    in_maps = _make_in_maps(x, consts)
    res = run_bass_kernel_spmd(
        nc, in_maps, core_ids=list(range(N_CORES)), trace=trace
    )

    out = np.empty((B, C, N), np.float32)
    for core in range(N_CORES):
        b, half = divmod(core, 2)
        sl = slice(half * NHALF, (half + 1) * NHALF)
        out[b, :, sl] = res.results[core]["out_t"].astype(np.float32).T \
            + x[b, :, sl]
    bt = _beta_term(
        np.asarray(ln_beta), np.asarray(expansion), np.asarray(reduction), a, q
    )
    if bt is not None:
        out += bt[None]
    if trace:
        kernel.last_results = res
    return out


# revision 24
# speedup vs baseline: 1.0632x; 1.0632x over previous
"""MultiHeadEMABlock Trainium2 kernel v6 (8-core SPMD, bass/Tile).

Math: out = x + R(EMA_bank(LN(x))) where the 8-head EMA+reduction kernel
bank K[d, tau] = sum_h rho[h,d] q_h^tau (rho = a*e*r*gamma) is numerically
rank-3 (q in [0.026, 0.57]; svals 36.7/5.3/0.83/0.13...). We compute
R=3 pseudo-heads: K ~= sum_r u_r[d] v_r(tau).

Per core (4 batches x 2 halves; halo W=128; q^128 ~ 1e-31 so no carry):
  chunk ck (128 timesteps):
   1. GpSimd: cast x f32->bf16
   2. PE: 4 ident-transpose matmuls -> x^T psum; +1-col matmuls -> Sx
   3. ACT: Square w/ accum_out -> Sx2 (scrap squares discarded)
   4. (per 2 chunks) LN finalize: rstd=exp(-.5 ln(var+eps)), negms=-m*rstd
   5. ACT: z^T = Identity(x^T * rstd + negms)  [per-partition affine]
   6. DVE/GpSimd: X_r = z^T * u_r (bf16 tensor_tensor, 3 ops)
   7. PE: 3 main T'_r matmuls (lower-tri v_r(i-j)) + 3 edge matmuls
      (last 32 rows of prev chunk's X_r, lags 1..63) accumulate in psum
   8. DVE: psum -> bf16 (per chunk pair), DMA out TIME-MAJOR [NHALF, C]
  Host: transpose back, add residual x and the data-independent beta term.
"""
import contextlib
import ctypes
import sys
import types

import numpy as np

for _p in ("/root/.axon_site/_ro/trn_rl_repo", "/opt/trn_rl_repo"):
    if _p not in sys.path:
        sys.path.append(_p)

B, C, N, H = 4, 512, 4096, 8
EPS = 1e-5
N_CORES = 8
NHALF = N // 2
W = 128
NW = NHALF + W
L = 128
NCH = NW // L  # 17
CT = C // 128  # 4
R = 3


# ---------------------------------------------------------------------------
# axon NTFF shim (lets run_bass_kernel_spmd(trace=True) capture HW profiles)
# ---------------------------------------------------------------------------
def _install_ntff_shim():
    if "antenv.axon_hooks" in sys.modules:
        return
    holder = {"hook": None}

    def _make(so_path):
        try:
            lib = ctypes.CDLL(so_path)
        except OSError:
            return None
        if not hasattr(lib, "axon_start_nrt_profile"):
            return None
        lib.axon_start_nrt_profile.argtypes = [
            ctypes.POINTER(ctypes.c_int64),
            ctypes.c_size_t,
        ]
        lib.axon_start_nrt_profile.restype = ctypes.c_int64
        lib.axon_stop_nrt_profile.argtypes = [ctypes.c_char_p]
        lib.axon_stop_nrt_profile.restype = ctypes.c_int64

        @contextlib.contextmanager
        def _hook(output_dir, device_ids):
            import jax

            jax.devices()
            if device_ids:
                ids = (ctypes.c_int64 * len(device_ids))(*device_ids)
                rc = lib.axon_start_nrt_profile(ids, len(device_ids))
            else:
                rc = lib.axon_start_nrt_profile(None, 0)
            if rc != 0:
                raise RuntimeError(f"axon_start_nrt_profile rc={rc}")
            try:
                yield
            finally:
                n = lib.axon_stop_nrt_profile(str(output_dir).encode())
                print(f"ntff profile: {n} file(s) -> {output_dir}", file=sys.stderr)

        return _hook

    mod = types.ModuleType("antenv.axon_hooks")
    mod.set_axon_ntff_profile_hook = lambda h: holder.__setitem__("hook", h)
    mod.get_axon_ntff_profile_hook = lambda: holder["hook"]
    sys.modules["antenv.axon_hooks"] = mod
    try:
        import antenv

        antenv.axon_hooks = mod
    except ImportError:
        pass
    holder["hook"] = _make("/opt/axon/libaxon_pjrt.so")


def _split_multiwait(nc, max_waits=1):
    """This walrus build rejects >1 sync wait per instruction; split extras
    onto same-engine NoOps inserted just before."""
    from concourse import mybir

    k = [0]
    for fn in nc.m.functions:
        for blk in fn.blocks:
            out = []
            for inst in blk.instructions:
                si = getattr(inst, "sync_info", None)
                if si is not None and len(si.on_wait) > max_waits:
                    waits = list(si.on_wait)
                    for w in waits[max_waits:]:
                        k[0] += 1
                        out.append(
                            mybir.InstNoOp(
                                name=f"{inst.name}-mw{k[0]}",
                                sync_info=mybir.SyncInfo(on_wait=[w], on_update=[]),
                                bass_nofuse=True,
                                engine=inst.engine,
                            )
                        )
                    inst.sync_info = mybir.SyncInfo(
                        on_wait=waits[:max_waits], on_update=list(si.on_update)
                    )
                out.append(inst)
            blk.instructions[:] = out


# ---------------------------------------------------------------------------
# program builder
# ---------------------------------------------------------------------------
def build_program():
    import concourse.bass as bass
    import concourse.tile as tile
    from concourse import mybir

    f32 = mybir.dt.float32
    bf16 = mybir.dt.bfloat16
    Op = mybir.AluOpType
    Act = mybir.ActivationFunctionType

    # 512-wide input DMA slices
    stat_slices = []
    o = 0
    while o < NW:
        w = min(512, NW - o)
        stat_slices.append((o, w))
        o += w

    # stat finalize groups (pairs)
    groups = [(k, min(k + 2, NCH)) for k in range(0, NCH, 2)]
    group_end = {g1 - 1: (g0, g1) for g0, g1 in groups}

    nc = bass.Bass(
        "TRN2",
        target_bir_lowering=False,
        debug=False,
        enable_asserts=False,
        num_devices=N_CORES,
    )
    xs_d = nc.dram_tensor("xs", [C, NW], f32, kind="ExternalInput").ap()
    tm_d = nc.dram_tensor("tmat", [R * 128, 128], bf16, kind="ExternalInput").ap()
    te_d = nc.dram_tensor("temat", [R * 32, 32], bf16, kind="ExternalInput").ap()
    ub_d = nc.dram_tensor("ubc", [R * 128, 512], bf16, kind="ExternalInput").ap()
    id_d = nc.dram_tensor("ident", [128, 128], bf16, kind="ExternalInput").ap()
    oc_d = nc.dram_tensor("onecol", [128, 1], bf16, kind="ExternalInput").ap()
    out_d = nc.dram_tensor("out_t", [NHALF, C], bf16, kind="ExternalOutput").ap()

    with tile.TileContext(nc) as tc:
        with contextlib.ExitStack() as ctx:
            pers = ctx.enter_context(tc.tile_pool(name="pers", bufs=1))
            xb_pool = ctx.enter_context(tc.tile_pool(name="xbp", bufs=4))
            zx_pool = ctx.enter_context(tc.tile_pool(name="zxp", bufs=3))
            ep_pool = ctx.enter_context(tc.tile_pool(name="epp", bufs=2))
            sq_pool = ctx.enter_context(tc.tile_pool(name="sqp", bufs=2))
            osb_pool = ctx.enter_context(tc.tile_pool(name="osbp", bufs=3))
            ps_t = ctx.enter_context(tc.tile_pool(name="pst", bufs=5, space="PSUM"))
            ps_o = ctx.enter_context(tc.tile_pool(name="pso", bufs=2, space="PSUM"))
            ps_s = ctx.enter_context(tc.tile_pool(name="pss", bufs=1, space="PSUM"))

            # ---- constants (scalar HWDGE queue; input on sync queue) ----
            ident = pers.tile([128, 128], bf16, tag="ident")
            nc.scalar.dma_start(out=ident[:], in_=id_d)
            onec = pers.tile([128, 1], bf16, tag="onec")
            nc.scalar.dma_start(out=onec[:], in_=oc_d)
            tm_t = [pers.tile([128, 128], bf16, tag=f"tm{r}", name=f"tm{r}")
                    for r in range(R)]
            for r in range(R):
                nc.scalar.dma_start(out=tm_t[r][:], in_=tm_d[r * 128:(r + 1) * 128, :])
            # packed edge stationary [3*32, 32] at partitions 0:96
            teP = pers.tile([96, 32], bf16, tag="teP")
            nc.scalar.dma_start(out=teP[:], in_=te_d)
            ub_t = [pers.tile([128, 512], bf16, tag=f"ub{r}", name=f"ub{r}")
                    for r in range(R)]
            for r in range(R):
                nc.scalar.dma_start(out=ub_t[r][:], in_=ub_d[r * 128:(r + 1) * 128, :])
            epsb = pers.tile([128, 1], f32, tag="eps")
            nc.gpsimd.memset(epsb[:], EPS)
            warm = pers.tile([128, 1], f32, tag="warm")
            nc.scalar.activation(out=warm[:], in_=epsb[:], func=Act.Square)

            s2sb = pers.tile([128, NCH], f32, tag="s2sb")
            m2s = pers.tile([128, NCH], f32, tag="m2s")
            var_a = pers.tile([128, NCH], f32, tag="var")
            lnv_a = pers.tile([128, NCH], f32, tag="lnv")
            rstd_a = pers.tile([128, NCH], f32, tag="rstd")
            negm_a = pers.tile([128, NCH], f32, tag="negm")

            # ---- input: slice 0 split per chunk (ramp priority), rest 512 ----
            xsl = {}
            for si, (o, wd) in enumerate(stat_slices):
                t = pers.tile([128, CT * 512], f32, tag=f"xsl{si}",
                              name=f"xsl{si}")
                xsl[si] = t
                if si == 0:
                    for cc in range(4):
                        for dt in range(CT):
                            nc.sync.dma_start(
                                out=t[:, dt * 512 + cc * L: dt * 512 + (cc + 1) * L],
                                in_=xs_d[dt * 128:(dt + 1) * 128,
                                         cc * L:(cc + 1) * L],
                            )
                else:
                    for dt in range(CT):
                        nc.sync.dma_start(
                            out=t[:, dt * 512: dt * 512 + wd],
                            in_=xs_d[dt * 128:(dt + 1) * 128, o:o + wd],
                        )

            def phase1(g0, g1):
                """cast, transpose, stat sums for chunks [g0, g1)"""
                out = {}
                statp = state.setdefault(
                    "statp", ps_s.tile([128, 32], f32, tag="stat", name="statp"))
                for ck in range(g0, g1):
                    xb = xb_pool.tile([128, 512], bf16, tag="xb")
                    si, co = divmod(ck * L, 512)
                    nc.vector.tensor_scalar(
                        out=xb[:].rearrange("p (dt i) -> p dt i", dt=CT),
                        in0=xsl[si][:].rearrange("p (dt n) -> p dt n", dt=CT)[
                            :, :, co:co + L],
                        scalar1=1.0, scalar2=None, op0=Op.mult,
                    )
                    xt = ps_t.tile([128, 512], f32, tag="xt")
                    for dt in range(CT):
                        nc.tensor.matmul(
                            out=xt[:, dt * 128:(dt + 1) * 128],
                            lhsT=xb[:, dt * 128:(dt + 1) * 128], rhs=ident[:],
                            start=True, stop=True,
                        )
                    for dt in range(CT):
                        nc.tensor.matmul(
                            out=statp[:, ck:ck + 1],
                            lhsT=xb[:, dt * 128:(dt + 1) * 128], rhs=onec[:],
                            start=(dt == 0), stop=(dt == CT - 1),
                        )
                    out[ck] = xt
                return out, statp

            def phase1b(g0, g1, xts):
                for ck in range(g0, g1):
                    scr = sq_pool.tile([128, 512], bf16, tag="scr")
                    nc.scalar.activation(
                        out=scr[:], in_=xts[ck][:], func=Act.Square,
                        accum_out=s2sb[:, ck:ck + 1],
                    )

            def finalize(g0, g1, statp):
                nc.scalar.activation(
                    out=m2s[:, g0:g1], in_=statp[:, g0:g1], func=Act.Square,
                    scale=1.0 / C,
                )
                nc.vector.scalar_tensor_tensor(
                    out=var_a[:, g0:g1], in0=s2sb[:, g0:g1], scalar=1.0 / C,
                    in1=m2s[:, g0:g1], op0=Op.mult, op1=Op.subtract,
                )
                nc.scalar.activation(
                    out=lnv_a[:, g0:g1], in_=var_a[:, g0:g1], func=Act.Ln,
                    bias=epsb[:],
                )
                nc.scalar.activation(
                    out=rstd_a[:, g0:g1], in_=lnv_a[:, g0:g1], func=Act.Exp,
                    scale=-0.5,
                )
                nc.vector.scalar_tensor_tensor(
                    out=negm_a[:, g0:g1], in0=statp[:, g0:g1], scalar=-1.0 / C,
                    in1=rstd_a[:, g0:g1], op0=Op.mult, op1=Op.mult,
                )

            state = {"xc_prev": None, "ep_prev": None}

            def phase2(g0, g1, xts):
                for ck in range(g0, g1):
                    xt = xts[ck]
                    zT = zx_pool.tile([128, 512], bf16, tag="zt")
                    nc.scalar.activation(
                        out=zT[:], in_=xt[:], func=Act.Identity,
                        scale=rstd_a[:, ck:ck + 1], bias=negm_a[:, ck:ck + 1],
                    )
                    x1 = zx_pool.tile([128, 512], bf16, tag="x1")
                    nc.vector.tensor_tensor(out=x1[:], in0=zT[:], in1=ub_t[0][:],
                                            op=Op.mult)
                    x2 = zx_pool.tile([128, 512], bf16, tag="x2")
                    nc.vector.tensor_tensor(out=x2[:], in0=zT[:], in1=ub_t[1][:],
                                            op=Op.mult)
                    x3 = zx_pool.tile([128, 512], bf16, tag="x3")
                    nc.gpsimd.tensor_tensor(out=x3[:], in0=zT[:], in1=ub_t[2][:],
                                            op=Op.mult)
                    xc = [x1, x2, x3]
                    # pack last 32 rows of X_r into partitions [32r, 32r+32)
                    epk = ep_pool.tile([96, 512], bf16, tag="epk")
                    for r in range(R):
                        nc.sync.dma_start(out=epk[r * 32:(r + 1) * 32, :],
                                          in_=xc[r][96:128, :])
                    if ck >= 1:
                        op_ps = ps_o.tile([128, 512], f32, tag="op")
                        nc.tensor.matmul(
                            out=op_ps[:], lhsT=tm_t[0][:], rhs=xc[0][:],
                            start=True, stop=False,
                        )
                        nc.tensor.matmul(
                            out=op_ps[0:32, :], lhsT=teP[:],
                            rhs=state["ep_prev"][:],
                            start=False, stop=False, skip_group_check=True,
                        )
                        nc.tensor.matmul(
                            out=op_ps[:], lhsT=tm_t[1][:], rhs=xc[1][:],
                            start=False, stop=False,
                        )
                        nc.tensor.matmul(
                            out=op_ps[:], lhsT=tm_t[2][:], rhs=xc[2][:],
                            start=False, stop=True,
                        )
                        mo = ck - 1
                        osb = osb_pool.tile([128, 512], bf16, tag="osb")
                        if ck % 2 == 0:
                            nc.scalar.activation(out=osb[:], in_=op_ps[:],
                                                 func=Act.Copy)
                        else:
                            nc.vector.tensor_scalar(
                                out=osb[:], in0=op_ps[:],
                                scalar1=1.0, scalar2=None, op0=Op.mult,
                            )
                        nc.sync.dma_start(
                            out=out_d[mo * 128:(mo + 1) * 128, :], in_=osb[:])
                    state["xc_prev"] = xc
                    state["ep_prev"] = epk

            # software-pipelined group loop
            prev = None
            for gi, (g0, g1) in enumerate(groups):
                if gi == 0:
                    cur = phase1(g0, g1)
                    phase1b(g0, g1, cur[0])
                    finalize(g0, g1, cur[1])
                    phase2(g0, g1, cur[0])
                    continue
                if prev is not None:
                    pg0, pg1, pxts, pstat = prev
                    finalize(pg0, pg1, pstat)
                    cur = phase1(g0, g1)
                    phase2(pg0, pg1, pxts)
                    phase1b(g0, g1, cur[0])
                else:
                    cur = phase1(g0, g1)
                    phase1b(g0, g1, cur[0])
                prev = (g0, g1, cur[0], cur[1])
            pg0, pg1, pxts, pstat = prev
            finalize(pg0, pg1, pstat)
            phase2(pg0, pg1, pxts)
    return nc


# ---------------------------------------------------------------------------
# host-side parameter prep
# ---------------------------------------------------------------------------
def _host_params(ln_gamma, ln_beta, expansion, reduction, alphas, dampen_factors):
    import ml_dtypes

    bf = ml_dtypes.bfloat16
    a = 1.0 / (1.0 + np.exp(-alphas.astype(np.float64)))
    q = (1.0 - a) / (1.0 + np.exp(-dampen_factors.astype(np.float64)))
    rho = (
        a[:, None]
        * expansion.astype(np.float64)
        * reduction.astype(np.float64)
        * ln_gamma.astype(np.float64)[None, :]
    )  # [H, C]
    tau = np.arange(L)
    K = rho.T @ (q[:, None] ** tau[None, :])  # [C, L]
    U, S, Vt = np.linalg.svd(K, full_matrices=False)
    u = U[:, :R] * S[None, :R]
    v = Vt[:R].copy()
    for r in range(R):
        s = np.sqrt(np.abs(u[:, r]).max() / max(np.abs(v[r]).max(), 1e-30))
        v[r] *= s
        u[:, r] /= s
    vpad = np.concatenate([v, np.zeros((R, 64))], 1)
    tmat = np.zeros((R * 128, 128), bf)
    for r in range(R):
        M = np.where(
            tau[:, None] <= tau[None, :],
            np.take(v[r], np.maximum(tau[None, :] - tau[:, None], 0)), 0.0,
        )  # [j, i] = v_r(i-j), i>=j
        tmat[r * 128:(r + 1) * 128, :] = M.astype(bf)
    jj = np.arange(32)
    lag = jj[None, :] + 32 - jj[:, None]  # [jj, i] in [1, 63]
    temat = np.zeros((R * 32, 32), bf)
    for r in range(R):
        temat[r * 32:(r + 1) * 32, :] = np.take(vpad[r], lag).astype(bf)
    ubc = np.zeros((R * 128, 512), bf)
    for r in range(R):
        ubc[r * 128:(r + 1) * 128, :] = np.tile(
            u[:, r].astype(bf)[None, :], (128, 1))
    consts = dict(
        tmat=tmat, temat=temat, ubc=ubc,
        ident=np.eye(128, dtype=bf),
        onecol=np.ones((128, 1), bf),
    )
    return a, q, consts


def _beta_term(ln_beta, expansion, reduction, a, q):
    if not np.any(ln_beta):
        return None
    n_idx = np.arange(N, dtype=np.float64)
    Cn = a[:, None] * (1.0 - q[:, None] ** (n_idx[None, :] + 1.0)) / (1.0 - q[:, None])
    w = (
        expansion.astype(np.float64)
        * reduction.astype(np.float64)
        * ln_beta.astype(np.float64)[None, :]
    )
    return np.einsum("hc,hn->cn", w, Cn).astype(np.float32)


def _make_in_maps(x, consts):
    in_maps = []
    for core in range(N_CORES):
        b, half = divmod(core, 2)
        xs = np.zeros((C, NW), np.float32)
        s = half * NHALF - W
        if s < 0:
            xs[:, W:] = x[b, :, :NHALF]
        else:
            xs[:] = x[b, :, s:s + NW]
        in_maps.append(dict(consts, xs=xs))
    return in_maps


def kernel(x, ln_gamma, ln_beta, expansion, reduction, alphas, dampen_factors,
           trace=False):
    _install_ntff_shim()
    from concourse.bass_utils import run_bass_kernel_spmd
    from concourse.bass_interp import get_hw_module

    x = np.asarray(x, np.float32)
    a, q, consts = _host_params(
        np.asarray(ln_gamma), np.asarray(ln_beta), np.asarray(expansion),
        np.asarray(reduction), np.asarray(alphas), np.asarray(dampen_factors),
    )
    nc = build_program()
    _split_multiwait(nc)
    nc.m = get_hw_module(nc.m)

    in_maps = _make_in_maps(x, consts)
    res = run_bass_kernel_spmd(
        nc, in_maps, core_ids=list(range(N_CORES)), trace=trace
    )

    out = np.empty((B, C, N), np.float32)
    for core in range(N_CORES):
        b, half = divmod(core, 2)
        sl = slice(half * NHALF, (half + 1) * NHALF)
        out[b, :, sl] = res.results[core]["out_t"].astype(np.float32).T \
            + x[b, :, sl]
    bt = _beta_term(
        np.asarray(ln_beta), np.asarray(expansion), np.asarray(reduction), a, q
    )
    if bt is not None:
        out += bt[None]
    if trace:
        kernel.last_results = res
    return out
